# revision 14
# baseline (speedup 1.0000x reference)
"""Trainium2 Bass kernel for nn_CNN_2D_Decoder (MoE per-camera decoder).

Math (per sample b with expert e = cam[b]):
  h1[t,o,p,q] = relu(sum_f x[b,f,t] * W1[e,f,o,p,q] + b1[e,o])          (o=128, pq=12)
  h2[t,o2,rs,pq] = relu(sum_o h1[t,o,p,q] * W2[e,o,o2,r,s] + b2[e,o2]) (o2=64, rs=12)
  out[t,h,w] = sigmoid(sum_o2 W3[e,o2] * h2[...] + b3[e]),  h=3p+r, w=4q+s

Strategy: group samples by expert on the host (kernel() sees the full
input), split each expert's samples into fixed-capacity chunks, and
distribute chunks over the 8 cores (SPMD, identical program; per-core
packed operand arrays). All matmuls run in float32r (TF32-like) at
1 cycle/row. Layer weights are the stationary operand so all samples
of a chunk share them; ScalarE fuses bias+relu (and bias+sigmoid)
directly out of PSUM. Layer-3 (the 64->1 conv) is folded into a
reduction matrix R that also applies W3, accumulated across the 6
partition-chunks of h2 in one PSUM tile.
"""
import math
import sys
import time

sys.path.insert(0, "/opt/trn_rl_repo")

import ml_dtypes
import numpy as np

import concourse.bass as bass
import concourse.mybir as mybir
import concourse.tile as tile
from concourse import bacc
from concourse.bass_utils import run_bass_kernel_spmd

B, F, T, C = 128, 512, 60, 15
H1, H2 = 128, 64
NCORES = 8
KCH = F // 128          # 4 k-chunks of the F contraction
PQ = 12                 # 3*4 first-conv spatial positions
MCH = 6                 # 768 / 128 partition chunks of (rs, o2)
F32R = mybir.dt.float32r

_cache = {}
LAST_EXEC_WALL_NS = None


def _build_nc(S, N):
    """Bass program: S chunks per core, N columns (= cap*60) per chunk."""
    nc = bacc.Bacc("TRN2", target_bir_lowering=False, debug=False)
    dt32 = mybir.dt.float32

    xd = nc.dram_tensor("xp", (S, KCH, 128, N), F32R, kind="ExternalInput").ap()
    w1d = nc.dram_tensor("w1p", (S, 128, KCH, PQ, 128), F32R, kind="ExternalInput").ap()
    w2d = nc.dram_tensor("w2p", (S, 128, MCH * 128), F32R, kind="ExternalInput").ap()
    rd = nc.dram_tensor("rp", (S, 128, MCH, PQ), mybir.dt.bfloat16, kind="ExternalInput").ap()
    b1d = nc.dram_tensor("b1p", (S, 128, 1), dt32, kind="ExternalInput").ap()
    b2d = nc.dram_tensor("b2p", (S, MCH, 128, 1), dt32, kind="ExternalInput").ap()
    b3d = nc.dram_tensor("b3p", (S, 128, 1), dt32, kind="ExternalInput").ap()
    od = nc.dram_tensor("out", (S, PQ // 4, 128, N), dt32, kind="ExternalOutput").ap()

    with tile.TileContext(nc) as tc:
        with (
            tc.tile_pool(name="wpool", bufs=2) as wpool,
            tc.tile_pool(name="xpool", bufs=2) as xpool,
            tc.tile_pool(name="bpool", bufs=2) as bpool,
            tc.tile_pool(name="h1pool", bufs=6) as h1pool,
            tc.tile_pool(name="h2pool", bufs=6) as h2pool,
            tc.tile_pool(name="opool", bufs=2) as opool,
            tc.tile_pool(name="ps1", bufs=2, space="PSUM") as ps1,
            tc.tile_pool(name="ps2", bufs=4, space="PSUM") as ps2,
            tc.tile_pool(name="ps3", bufs=2, space="PSUM") as ps3,
        ):
            for s in range(S):
                w1t = wpool.tile([128, KCH, PQ, 128], F32R, tag="w1")
                w2t = wpool.tile([128, MCH * 128], F32R, tag="w2")
                rt = wpool.tile([128, MCH, PQ], mybir.dt.bfloat16, tag="r")
                b1t = bpool.tile([128, 1], dt32, tag="b1")
                b2t = bpool.tile([128, MCH], dt32, tag="b2")
                b3t = bpool.tile([128, 1], dt32, tag="b3")
                # DMAs in (approximate) consumption order: tiny biases first,
                # then the k0..k3 W1 slabs interleaved with the X loads (so the
                # first L1 matmuls wait on ~0.3 MB, not the full chunk), then
                # W2/R (first L2/L3), then the remaining W1 slabs.
                nc.sync.dma_start(out=b1t, in_=b1d[s])
                nc.sync.dma_start(out=b2t, in_=b2d[s].rearrange("m p one -> p (m one)"))
                nc.sync.dma_start(out=b3t, in_=b3d[s])
                xts = []
                for k in range(KCH):
                    nc.sync.dma_start(out=w1t[:, k, 0:3], in_=w1d[s, :, k, 0:3])
                    xt = xpool.tile([128, N], F32R, tag=f"x{k}")
                    nc.sync.dma_start(out=xt, in_=xd[s, k])
                    xts.append(xt)
                nc.sync.dma_start(out=w2t[:, 0:256], in_=w2d[s, :, 0:256])
                nc.sync.dma_start(out=rt, in_=rd[s])
                nc.sync.dma_start(out=w2t[:, 256:768], in_=w2d[s, :, 256:768])
                for j in range(1, 4):
                    for k in range(KCH):
                        nc.sync.dma_start(
                            out=w1t[:, k, 3 * j : 3 * (j + 1)],
                            in_=w1d[s, :, k, 3 * j : 3 * (j + 1)],
                        )

                for batch in range(PQ // 4):
                    h1s = []
                    for g in range(4):
                        pq = 4 * batch + g
                        p1 = ps1.tile([128, N], dt32, tag="p1")
                        for k in range(KCH):
                            nc.tensor.matmul(
                                p1[:],
                                w1t[:, k, pq, :],
                                xts[k][:],
                                start=(k == 0),
                                stop=(k == KCH - 1),
                            )
                        h1t = h1pool.tile([128, N], F32R, tag="h1")
                        nc.scalar.activation(
                            out=h1t[:], in_=p1[:],
                            func=mybir.ActivationFunctionType.Relu, bias=b1t[:],
                        )
                        h1s.append(h1t)
                    p3 = ps3.tile([128, N], dt32, tag="p3")
                    for m in range(MCH):
                        h2s = []
                        for g in range(4):
                            p2 = ps2.tile([128, N], dt32, tag="p2")
                            nc.tensor.matmul(
                                p2[:],
                                w2t[:, bass.ts(m, 128)],
                                h1s[g][:],
                                start=True, stop=True,
                            )
                            h2t = h2pool.tile([128, N], mybir.dt.bfloat16, tag="h2")
                            if (batch * 24 + m * 4 + g) % 3 == 0:
                                # 1/3 of the bias+relu passes on ScalarE ...
                                nc.scalar.activation(
                                    out=h2t[:], in_=p2[:],
                                    func=mybir.ActivationFunctionType.Relu,
                                    bias=b2t[:, m : m + 1],
                                )
                            else:
                                # ... and 2/3 on the otherwise-idle VectorE
                                nc.vector.tensor_scalar(
                                    out=h2t[:], in0=p2[:],
                                    scalar1=b2t[:, m : m + 1], scalar2=0.0,
                                    op0=mybir.AluOpType.add, op1=mybir.AluOpType.max,
                                )
                            h2s.append(h2t)
                        # 4 narrow (M=12) reductions issued back-to-back into
                        # distinct PE column groups -> they run concurrently
                        for g in range(4):
                            nc.tensor.matmul(
                                p3[32 * g : 32 * g + PQ, :],
                                rt[:, m, :],
                                h2s[g][:],
                                start=(m == 0), stop=(m == MCH - 1),
                                tile_position=(0, 32 * g),
                            )
                    ot = opool.tile([128, N], dt32, tag="o")
                    nc.scalar.activation(
                        out=ot[:], in_=p3[:],
                        func=mybir.ActivationFunctionType.Sigmoid, bias=b3t[:],
                    )
                    nc.sync.dma_start(out=od[s, batch], in_=ot)
    nc.compile()
    return nc


def _get_nc(S, N):
    key = (S, N)
    if key not in _cache:
        _cache[key] = _build_nc(S, N)
    return _cache[key]


def _pack(x, cam, W1, b1, W2, b2, W3, b3):
    x = np.asarray(x, dtype=np.float32)
    cam = np.asarray(cam).astype(np.int64)
    W1 = np.asarray(W1, dtype=np.float32)
    b1 = np.asarray(b1, dtype=np.float32)
    W2 = np.asarray(W2, dtype=np.float32)
    b2 = np.asarray(b2, dtype=np.float32)
    W3 = np.asarray(W3, dtype=np.float32)
    b3 = np.asarray(b3, dtype=np.float32)

    counts = np.bincount(cam, minlength=C)
    order = np.argsort(cam, kind="stable")

    # pick chunk capacity (samples per chunk); N = cap*60 must be in [256, 512]
    best = None
    for cap in range(5, 9):
        Q = int(sum(math.ceil(n / cap) for n in counts if n > 0))
        S = max(1, math.ceil(Q / NCORES))
        cost = (S * cap, cap)  # fewer padded sample-slots; tie -> smaller cap
        if best is None or cost < best[0]:
            best = (cost, cap, S, Q)
    _, cap, S, Q = best
    N = cap * T

    # chunk list: (expert, sample_indices)
    chunks = []
    off = 0
    for e in range(C):
        n = int(counts[e])
        ids = order[off : off + n]
        off += n
        for i in range(0, n, cap):
            chunks.append((e, ids[i : i + cap]))
    assert len(chunks) == Q

    # per-core packed arrays
    xp = np.zeros((NCORES, S, KCH, 128, N), np.float32)
    w1p = np.zeros((NCORES, S, 128, KCH, PQ, 128), np.float32)
    w2p = np.zeros((NCORES, S, 128, MCH * 128), np.float32)
    rp = np.zeros((NCORES, S, 128, MCH, PQ), np.float32)
    b1p = np.zeros((NCORES, S, 128, 1), np.float32)
    b2p = np.zeros((NCORES, S, MCH, 128, 1), np.float32)
    b3p = np.zeros((NCORES, S, 128, 1), np.float32)

    # base reduction matrix: R3[m, 64a+o2, 2m+a] = 1
    R3 = np.zeros((MCH, 128, PQ), np.float32)
    for m in range(MCH):
        for a2 in range(2):
            R3[m, 64 * a2 : 64 * (a2 + 1), 2 * m + a2] = 1.0

    # W2 rearranged to (i, rs*64+o2)
    W2r = W2.transpose(0, 1, 3, 4, 2).reshape(C, H1, PQ * H2)
    # W1 rearranged to (f_local partitions, k, pq, o)
    W1r = W1.reshape(C, KCH, 128, H1, 3, 4).transpose(0, 2, 1, 4, 5, 3).reshape(
        C, 128, KCH, PQ, H1
    )

    assign = []  # (core, slot, expert, ids)
    for ci, ch in enumerate(chunks):
        core, slot = ci % NCORES, ci // NCORES
        e, ids = ch
        assign.append((core, slot, e, ids))
        w1p[core, slot] = W1r[e]
        w2p[core, slot] = W2r[e]
        rp[core, slot] = (R3 * np.tile(W3[e], 2)[None, :, None]).transpose(1, 0, 2)
        b1p[core, slot, :, 0] = b1[e]
        b2p[core, slot, :, :, 0] = np.tile(b2[e], 2).reshape(1, 128)
        b3p[core, slot, :, 0] = b3[e]
        # x columns: sample-major, (f, cap*T)
        xs = x[ids]  # (n, F, T)
        ncols = len(ids) * T
        xp[core, slot, :, :, :ncols] = (
            xs.transpose(1, 0, 2).reshape(F, ncols).reshape(KCH, 128, ncols)
        )

    nc = _get_nc(S, N)
    in_maps = [
        {
            "xp": np.ascontiguousarray(xp[c]),
            "w1p": np.ascontiguousarray(w1p[c]),
            "w2p": np.ascontiguousarray(w2p[c]),
            "rp": np.ascontiguousarray(rp[c]).astype(ml_dtypes.bfloat16),
            "b1p": np.ascontiguousarray(b1p[c]),
            "b2p": np.ascontiguousarray(b2p[c]),
            "b3p": np.ascontiguousarray(b3p[c]),
        }
        for c in range(NCORES)
    ]
    return nc, in_maps, assign, S, N


def _unpack(results, assign):
    out = np.empty((B, T, 9, 16), np.float32)
    for core, slot, e, ids in assign:
        oc = results[core]["out"][slot]  # (3 batches, 128, N)
        ncols = len(ids) * T
        arr = np.stack(
            [
                oc[pq // 4, 32 * (pq % 4) : 32 * (pq % 4) + PQ, :ncols]
                for pq in range(PQ)
            ]
        )  # (pq, j, ncols)
        arr = arr.reshape(3, 4, 3, 4, len(ids), T)
        # [p, q, r, s, n, t] -> [n, t, (3p+r), (4q+s)]
        arr = arr.transpose(4, 5, 0, 2, 1, 3).reshape(len(ids), T, 9, 16)
        out[ids] = arr
    return out


def kernel(x, cam, W1, b1, W2, b2, W3, b3):
    global LAST_EXEC_WALL_NS
    nc, in_maps, assign, S, N = _pack(x, cam, W1, b1, W2, b2, W3, b3)
    t0 = time.perf_counter_ns()
    res = run_bass_kernel_spmd(nc, in_maps, core_ids=list(range(NCORES)))
    LAST_EXEC_WALL_NS = time.perf_counter_ns() - t0
    return _unpack(res.results, assign)


# revision 30
# speedup vs baseline: 1.0094x; 1.0094x over previous
"""Trainium2 Bass kernel for nn_CNN_2D_Decoder (MoE per-camera decoder).

Math (per sample b with expert e = cam[b]):
  h1[t,o,p,q] = relu(sum_f x[b,f,t] * W1[e,f,o,p,q] + b1[e,o])          (o=128, pq=12)
  h2[t,o2,rs,pq] = relu(sum_o h1[t,o,p,q] * W2[e,o,o2,r,s] + b2[e,o2]) (o2=64, rs=12)
  out[t,h,w] = sigmoid(sum_o2 W3[e,o2] * h2[...] + b3[e]),  h=3p+r, w=4q+s

Strategy: group samples by expert on the host (kernel() sees the full
input), split each expert's samples into fixed-capacity chunks, and
distribute chunks over the 8 cores (SPMD, identical program; per-core
packed operand arrays). All matmuls run in float32r (TF32-like) at
1 cycle/row. Layer weights are the stationary operand so all samples
of a chunk share them; ScalarE fuses bias+relu (and bias+sigmoid)
directly out of PSUM. Layer-3 (the 64->1 conv) is folded into a
reduction matrix R that also applies W3, accumulated across the 6
partition-chunks of h2 in one PSUM tile.
"""
import math
import sys
import time

sys.path.insert(0, "/opt/trn_rl_repo")

import ml_dtypes
import numpy as np

import concourse.bass as bass
import concourse.mybir as mybir
import concourse.tile as tile
from concourse import bacc
from concourse.bass_utils import run_bass_kernel_spmd

B, F, T, C = 128, 512, 60, 15
H1, H2 = 128, 64
NCORES = 8
KCH = F // 128          # 4 k-chunks of the F contraction
PQ = 12                 # 3*4 first-conv spatial positions
MCH = 6                 # 768 / 128 partition chunks of (rs, o2)
F32R = mybir.dt.float32r

_cache = {}
LAST_EXEC_WALL_NS = None


def _build_nc(sizes):
    """Bass program: len(sizes) chunks per core; slot i spans sizes[i]
    matmul columns (one column = one (sample, t) pair; samples may split
    across slots). Same program on all 8 cores."""
    S = len(sizes)
    N = max(sizes)
    Ns = list(sizes)
    nc = bacc.Bacc("TRN2", target_bir_lowering=False, debug=False)
    dt32 = mybir.dt.float32

    xd = nc.dram_tensor("xp", (S, KCH, 128, N), F32R, kind="ExternalInput").ap()
    w1d = nc.dram_tensor("w1p", (S, 128, KCH, PQ, 128), F32R, kind="ExternalInput").ap()
    w2d = nc.dram_tensor("w2p", (S, 128, MCH * 128), F32R, kind="ExternalInput").ap()
    rd = nc.dram_tensor("rp", (S, 128, MCH, PQ), mybir.dt.bfloat16, kind="ExternalInput").ap()
    b1d = nc.dram_tensor("b1p", (S, 128, 1), dt32, kind="ExternalInput").ap()
    b2d = nc.dram_tensor("b2p", (S, MCH, 128, 1), dt32, kind="ExternalInput").ap()
    b3d = nc.dram_tensor("b3p", (S, 128, 1), dt32, kind="ExternalInput").ap()
    od = nc.dram_tensor("out", (S, PQ // 4, 128, N), dt32, kind="ExternalOutput").ap()

    with tile.TileContext(nc) as tc:
        with (
            tc.tile_pool(name="wpool", bufs=2) as wpool,
            tc.tile_pool(name="xpool", bufs=2) as xpool,
            tc.tile_pool(name="bpool", bufs=2) as bpool,
            tc.tile_pool(name="h1pool", bufs=6) as h1pool,
            tc.tile_pool(name="h2pool", bufs=6) as h2pool,
            tc.tile_pool(name="opool", bufs=2) as opool,
            tc.tile_pool(name="ps1", bufs=2, space="PSUM") as ps1,
            tc.tile_pool(name="ps2", bufs=4, space="PSUM") as ps2,
            tc.tile_pool(name="ps3", bufs=2, space="PSUM") as ps3,
        ):
            for s in range(S):
                Nc = Ns[s]
                w1t = wpool.tile([128, KCH, PQ, 128], F32R, tag="w1")
                w2t = wpool.tile([128, MCH * 128], F32R, tag="w2")
                rt = wpool.tile([128, MCH, PQ], mybir.dt.bfloat16, tag="r")
                b1t = bpool.tile([128, 1], dt32, tag="b1")
                b2t = bpool.tile([128, MCH], dt32, tag="b2")
                b3t = bpool.tile([128, 1], dt32, tag="b3")
                # DMAs in (approximate) consumption order: tiny biases first,
                # then the k0..k3 W1 slabs interleaved with the X loads (so the
                # first L1 matmuls wait on ~0.3 MB, not the full chunk), then
                # W2/R (first L2/L3), then the remaining W1 slabs.
                nc.sync.dma_start(out=b1t, in_=b1d[s])
                nc.sync.dma_start(out=b2t, in_=b2d[s].rearrange("m p one -> p (m one)"))
                nc.sync.dma_start(out=b3t, in_=b3d[s])
                xts = []
                for k in range(KCH):
                    nc.sync.dma_start(out=w1t[:, k, 0:3], in_=w1d[s, :, k, 0:3])
                    xt = xpool.tile([128, Nc], F32R, tag=f"x{k}")
                    nc.sync.dma_start(out=xt, in_=xd[s, k, :, 0:Nc])
                    xts.append(xt)
                nc.sync.dma_start(out=w2t[:, 0:256], in_=w2d[s, :, 0:256])
                nc.sync.dma_start(out=rt, in_=rd[s])
                nc.sync.dma_start(out=w2t[:, 256:768], in_=w2d[s, :, 256:768])
                for j in range(1, 4):
                    for k in range(KCH):
                        nc.sync.dma_start(
                            out=w1t[:, k, 3 * j : 3 * (j + 1)],
                            in_=w1d[s, :, k, 3 * j : 3 * (j + 1)],
                        )

                for batch in range(PQ // 4):
                    h1s = []
                    for g in range(4):
                        pq = 4 * batch + g
                        p1 = ps1.tile([128, Nc], dt32, tag="p1")
                        for k in range(KCH):
                            nc.tensor.matmul(
                                p1[:],
                                w1t[:, k, pq, :],
                                xts[k][:],
                                start=(k == 0),
                                stop=(k == KCH - 1),
                            )
                        h1t = h1pool.tile([128, Nc], F32R, tag="h1")
                        nc.scalar.activation(
                            out=h1t[:], in_=p1[:],
                            func=mybir.ActivationFunctionType.Relu, bias=b1t[:],
                        )
                        h1s.append(h1t)
                    p3 = ps3.tile([128, Nc], dt32, tag="p3")
                    for m in range(MCH):
                        h2s = []
                        for g in range(4):
                            p2 = ps2.tile([128, Nc], dt32, tag="p2")
                            nc.tensor.matmul(
                                p2[:],
                                w2t[:, bass.ts(m, 128)],
                                h1s[g][:],
                                start=True, stop=True,
                            )
                            h2t = h2pool.tile([128, Nc], mybir.dt.bfloat16, tag="h2")
                            if (batch * 24 + m * 4 + g) % 5 < 2:
                                # 40% of the bias+relu passes on ScalarE ...
                                nc.scalar.activation(
                                    out=h2t[:], in_=p2[:],
                                    func=mybir.ActivationFunctionType.Relu,
                                    bias=b2t[:, m : m + 1],
                                )
                            else:
                                # ... and 60% on the otherwise-idle VectorE
                                nc.vector.tensor_scalar(
                                    out=h2t[:], in0=p2[:],
                                    scalar1=b2t[:, m : m + 1], scalar2=0.0,
                                    op0=mybir.AluOpType.add, op1=mybir.AluOpType.max,
                                )
                            h2s.append(h2t)
                        # 4 narrow (M=12) reductions issued back-to-back into
                        # distinct PE column groups -> they run concurrently
                        for g in range(4):
                            nc.tensor.matmul(
                                p3[32 * g : 32 * g + PQ, :],
                                rt[:, m, :],
                                h2s[g][:],
                                start=(m == 0), stop=(m == MCH - 1),
                                tile_position=(0, 32 * g),
                            )
                    ot = opool.tile([128, Nc], dt32, tag="o")
                    nc.scalar.activation(
                        out=ot[:], in_=p3[:],
                        func=mybir.ActivationFunctionType.Sigmoid, bias=b3t[:],
                    )
                    nc.sync.dma_start(out=od[s, batch, :, 0:Nc], in_=ot)
    nc.compile()
    return nc


def _get_nc(sizes):
    key = tuple(sizes)
    if key not in _cache:
        _cache[key] = _build_nc(key)
    return _cache[key]


def _greedy_fill(sizes, ncols):
    """Assign expert column-counts to 8 copies of the per-core slot-size
    vector (sizes in columns). A slot holds columns of one expert only.
    Returns list of (core, slot, expert, take_cols) or None if infeasible."""
    slots = sorted(
        ((sizes[i], c, i) for i in range(len(sizes)) for c in range(NCORES)),
        reverse=True,
    )
    remaining = sorted(((int(n), e) for e, n in enumerate(ncols) if n > 0), reverse=True)
    out = []
    while remaining:
        remaining.sort(reverse=True)
        r, e = remaining.pop(0)
        if not slots:
            return None
        if r >= slots[0][0]:
            cap, core, idx = slots.pop(0)       # biggest slot, filled fully
            take = cap
        else:
            # smallest slot that fits the whole remainder (exact-fit-ish)
            j = len(slots) - 1
            while slots[j][0] < r:
                j -= 1
            cap, core, idx = slots.pop(j)
            take = r
        out.append((core, idx, e, take))
        if r - take > 0:
            remaining.append((r - take, e))
    return out


def _pack(x, cam, W1, b1, W2, b2, W3, b3):
    x = np.asarray(x, dtype=np.float32)
    cam = np.asarray(cam).astype(np.int64)
    W1 = np.asarray(W1, dtype=np.float32)
    b1 = np.asarray(b1, dtype=np.float32)
    W2 = np.asarray(W2, dtype=np.float32)
    b2 = np.asarray(b2, dtype=np.float32)
    W3 = np.asarray(W3, dtype=np.float32)
    b3 = np.asarray(b3, dtype=np.float32)

    counts = np.bincount(cam, minlength=C)
    order = np.argsort(cam, kind="stable")
    id_of = {}  # expert -> its sorted sample ids
    off = 0
    for e in range(C):
        id_of[e] = np.array(order[off : off + int(counts[e])], dtype=np.int64)
        off += int(counts[e])
    ncols = counts * T  # columns per expert (column = one (sample, t))

    # choose the per-core slot-size vector (sizes in columns; each slot
    # must keep the f32r matmuls in their fast regime => >= 300 cols)
    import itertools

    best = None
    size_opts = list(range(480, 299, -30))
    for S_ in range(2, 6):
        for sizes in itertools.combinations_with_replacement(size_opts, S_):
            fill = _greedy_fill(sizes, ncols)
            if fill is None:
                continue
            cost = (sum(sizes), S_)
            if best is None or cost < best[0]:
                best = (cost, sizes, fill)
    assert best is not None, "no feasible slot layout"
    _, sizes, fill = best
    global LAST_SIZES
    LAST_SIZES = sizes
    S = len(sizes)
    N = max(sizes)

    # chunk list: (core, slot, expert, col_start_in_expert_stream, ncols)
    chunks = []
    consumed = [0] * C
    for core, slot, e, take in fill:
        chunks.append((core, slot, e, consumed[e], take))
        consumed[e] += take

    # per-core packed arrays
    xp = np.zeros((NCORES, S, KCH, 128, N), np.float32)
    w1p = np.zeros((NCORES, S, 128, KCH, PQ, 128), np.float32)
    w2p = np.zeros((NCORES, S, 128, MCH * 128), np.float32)
    rp = np.zeros((NCORES, S, 128, MCH, PQ), np.float32)
    b1p = np.zeros((NCORES, S, 128, 1), np.float32)
    b2p = np.zeros((NCORES, S, MCH, 128, 1), np.float32)
    b3p = np.zeros((NCORES, S, 128, 1), np.float32)

    # base reduction matrix: R3[m, 64a+o2, 2m+a] = 1
    R3 = np.zeros((MCH, 128, PQ), np.float32)
    for m in range(MCH):
        for a2 in range(2):
            R3[m, 64 * a2 : 64 * (a2 + 1), 2 * m + a2] = 1.0

    # W2 rearranged to (i, rs*64+o2)
    W2r = W2.transpose(0, 1, 3, 4, 2).reshape(C, H1, PQ * H2)
    # W1 rearranged to (f_local partitions, k, pq, o)
    W1r = W1.reshape(C, KCH, 128, H1, 3, 4).transpose(0, 2, 1, 4, 5, 3).reshape(
        C, 128, KCH, PQ, H1
    )

    # per-expert column streams (f-major), cut into chunk column ranges
    xstream = {
        e: x[id_of[e]].transpose(1, 0, 2).reshape(KCH, 128, int(ncols[e]))
        for e in range(C)
        if ncols[e] > 0
    }
    for core, slot, e, a, n in chunks:
        w1p[core, slot] = W1r[e]
        w2p[core, slot] = W2r[e]
        rp[core, slot] = (R3 * np.tile(W3[e], 2)[None, :, None]).transpose(1, 0, 2)
        b1p[core, slot, :, 0] = b1[e]
        b2p[core, slot, :, :, 0] = np.tile(b2[e], 2).reshape(1, 128)
        b3p[core, slot, :, 0] = b3[e]
        xp[core, slot, :, :, :n] = xstream[e][:, :, a : a + n]
    assign = (chunks, id_of, ncols)

    nc = _get_nc(sizes)
    in_maps = [
        {
            "xp": np.ascontiguousarray(xp[c]),
            "w1p": np.ascontiguousarray(w1p[c]),
            "w2p": np.ascontiguousarray(w2p[c]),
            "rp": np.ascontiguousarray(rp[c]).astype(ml_dtypes.bfloat16),
            "b1p": np.ascontiguousarray(b1p[c]),
            "b2p": np.ascontiguousarray(b2p[c]),
            "b3p": np.ascontiguousarray(b3p[c]),
        }
        for c in range(NCORES)
    ]
    return nc, in_maps, assign, S, N


def _unpack(results, assign):
    chunks, id_of, ncols = assign
    streams = {
        e: np.empty((int(ncols[e]), 9, 16), np.float32)
        for e in range(C)
        if ncols[e] > 0
    }
    for core, slot, e, a, n in chunks:
        oc = results[core]["out"][slot]  # (3 batches, 128, N)
        arr = np.stack(
            [oc[pq // 4, 32 * (pq % 4) : 32 * (pq % 4) + PQ, :n] for pq in range(PQ)]
        )  # (pq, j, n)
        arr = arr.reshape(3, 4, 3, 4, n)
        # [p, q, r, s, col] -> [col, (3p+r), (4q+s)]
        arr = arr.transpose(4, 0, 2, 1, 3).reshape(n, 9, 16)
        streams[e][a : a + n] = arr
    out = np.empty((B, T, 9, 16), np.float32)
    for e, st in streams.items():
        out[id_of[e]] = st.reshape(-1, T, 9, 16)
    return out


def kernel(x, cam, W1, b1, W2, b2, W3, b3):
    global LAST_EXEC_WALL_NS
    nc, in_maps, assign, S, N = _pack(x, cam, W1, b1, W2, b2, W3, b3)
    t0 = time.perf_counter_ns()
    res = run_bass_kernel_spmd(nc, in_maps, core_ids=list(range(NCORES)))
    LAST_EXEC_WALL_NS = time.perf_counter_ns() - t0
    return _unpack(res.results, assign)
